# revision 25
# baseline (speedup 1.0000x reference)
"""Llama GQA attention (B=2,S=2048,H=32,KV=8,D=128,DM=4096) on 8 trn2 cores.

Sharding: DP=2 over sequences x TP=4 over heads. Core c = (b=c//4, g=c%4):
seq b's 2048 tokens, q-heads [8g,8g+8), kv-heads [2g,2g+2). Each core computes
its partial o-proj output; host sums the 4 TP partials per sequence.

Device layout trick: everything lives transposed ([feat, tok]) so the
contraction dim is always on partitions and no on-chip transposes are needed.
  qkv^T = W^T.T @ hidden^T          (W^T, hidden^T pre-transposed on host)
  S^T[j,i] = (k^T).T @ q^T          (contraction d=128 = one partition tile)
  P^T = exp(scale*S^T) * mask       (no max-subtraction: scores ~ N(0,1))
  C^T[d,i] = sum_j V[j,d].T ... accumulated as lhsT=V tile, rhs=P^T
  l via all-ones [128,128] lhsT matmul (row-sum broadcast to all
  partitions); 1/l = reciprocal on VectorE (off ACT's exp critical path);
  out^T = Wo^T.T @ (C^T * recip)
RoPE: rotate_half as a signed 128x128 permutation matmul + cos/sin elementwise.
All matmuls bf16 inputs, fp32 PSUM accumulation.

Perf structure (per trace analysis; ~1.13ms -> ~0.87ms):
- all emission is software-pipelined: engine queues are in-order, so the op
  that unblocks the next engine is always emitted before ops that wait
  (scores of group g+1 before AV of g; RoPE tail of ft one group late)
- diagonal attention tiles run on [128*r:IB] sub-ranges (causal triangle),
  with one [128,128] triangle mask; exp stays grouped (ACT call overhead
  costs more than the garbage columns it would skip)
- the softmax tail (reciprocal+mul) of block X is emitted after block X+1's
  first exp so ACT's FIFO never blocks the scores->exp->AV chain
- phase 1 double-buffers hb and rotates 4 QK PSUM banks; startup loads are
  chunked on the sync HWDGE ring so the first matmul starts at ~15us
"""

import numpy as np
import ml_dtypes

import concourse.bass as bass
import concourse.mybir as mybir
import concourse.tile as tile
from concourse.bass_utils import run_bass_kernel_spmd

F32 = mybir.dt.float32
BF16 = mybir.dt.bfloat16
BF = ml_dtypes.bfloat16


class Cfg:
    def __init__(self, S=2048, H=32, KV=8, D=128, TP=4, DP=2, TB=512, IB=512):
        self.S, self.H, self.KV, self.D = S, H, KV, D
        self.TP, self.DP = TP, DP
        self.DM = H * D
        self.HL = H // TP            # local q heads
        self.KVL = KV // TP          # local kv heads
        self.QF = self.HL * D        # local q feats
        self.KF = self.KVL * D
        self.VF = self.KVL * D
        self.LF = self.HL * D        # local o-proj contraction feats
        self.NKT = self.DM // 128    # K-tiles for qkv proj
        self.NQK = (self.QF + self.KF) // 128
        self.TB = min(TB, S)         # token block (qkv / o-proj moving dim)
        self.IB = min(IB, S)         # query block in attention
        self.ND = self.IB // 128     # j-tiles per i-block (diag patterns)
        self.GJ = 2 if self.ND >= 2 else 1   # j-tiles per exp group
        self.scale = float(D) ** -0.5


def build_kernel(tc, cfg):
    nc = tc.nc
    S, D = cfg.S, cfg.D
    TB, IB, ND, GJ = cfg.TB, cfg.IB, cfg.ND, cfg.GJ
    NKT, NQK = cfg.NKT, cfg.NQK
    NTB = S // TB
    NTT = TB // 128                  # tok tiles per block (for V)
    NIB = S // IB

    hid = nc.dram_tensor("hid_t", [cfg.DM, S], BF16, kind="ExternalInput").ap()
    wqk = nc.dram_tensor("wqk_t", [cfg.DM, cfg.QF + cfg.KF], BF16, kind="ExternalInput").ap()
    wv = nc.dram_tensor("wv_t", [cfg.DM, cfg.VF], BF16, kind="ExternalInput").ap()
    wo = nc.dram_tensor("wo_t", [cfg.LF, cfg.DM], BF16, kind="ExternalInput").ap()
    cos = nc.dram_tensor("cos_t", [128, S], F32, kind="ExternalInput").ap()
    sin = nc.dram_tensor("sin_t", [128, S], F32, kind="ExternalInput").ap()
    msk = nc.dram_tensor("masks", [128, 128], BF16, kind="ExternalInput").ap()
    rt = nc.dram_tensor("rt", [128, 128], BF16, kind="ExternalInput").ap()
    out = nc.dram_tensor("out_t", [cfg.DM, S], F32, kind="ExternalOutput").ap()

    hid_r = hid.rearrange("(a p) t -> p a t", p=128)
    wqk_r = wqk.rearrange("(a p) f -> p a f", p=128)
    wv_r = wv.rearrange("(a p) f -> p a f", p=128)
    wo_r = wo.rearrange("(a p) f -> p a f", p=128)

    with tc.tile_pool(name="res", bufs=1) as res:
        qkT = res.tile([128, NQK, S], BF16, tag="qkT")
        v_sb = res.tile([128, S // 128, cfg.VF], BF16, tag="v")
        cos_t = res.tile([128, S], F32, tag="cos")
        sin_t = res.tile([128, S], F32, tag="sin")
        msk_t = res.tile([128, 128], BF16, tag="msk")
        rt_t = res.tile([128, 128], BF16, tag="rt")
        ones128 = res.tile([128, 128], BF16, tag="ones128")

        # Startup: only the first token block's cos/sin rows load up front
        # (the rest loads from the Scalar HWDGE ring once phase 1 is rolling)
        # so hb/wt DMAs own the HBM bandwidth before the first matmul. These
        # ride the scalar ring so the sync ring leads with tb0's hb/wt chunks.
        nc.scalar.dma_start(cos_t[:, 0:TB], cos[:, 0:TB])
        nc.scalar.dma_start(sin_t[:, 0:TB], sin[:, 0:TB])
        nc.scalar.dma_start(msk_t[:], msk[:])
        nc.scalar.dma_start(rt_t[:], rt[:])
        nc.vector.memset(ones128[:], 1.0)

        # ---------------- Phase 1: fused QKV projection + RoPE ----------------
        with tc.tile_pool(name="p1res", bufs=1) as p1res, \
             tc.tile_pool(name="p1", bufs=3) as p1, \
             tc.tile_pool(name="p1h", bufs=2) as p1h, \
             tc.tile_pool(name="p1w", bufs=2) as p1w, \
             tc.tile_pool(name="ps_qk", bufs=4, space="PSUM") as ps_qk, \
             tc.tile_pool(name="ps_rot", bufs=2, space="PSUM") as ps_rot, \
             tc.tile_pool(name="ps_v", bufs=2, space="PSUM") as ps_v:
            wv_t = p1res.tile([128, NKT, cfg.VF], BF16, tag="wv")

            # RoPE tail of feature-tile ft is emitted one PE-group late, so
            # the rot matmul never heads the PE queue while ACT's raw copy
            # is still in flight.
            rope_pending = []

            def emit_rope():
                if not rope_pending:
                    return
                ps_p, raw_p, ft_p, ts_p = rope_pending.pop()
                rps = ps_rot.tile([128, TB], F32, tag="rps")
                nc.tensor.matmul(rps[:], rt_t[:], raw_p[:], start=True,
                                 stop=True)
                t1 = p1.tile([128, TB], BF16, tag="t1")
                nc.vector.tensor_mul(t1[:], ps_p[:], cos_t[:, ts_p])
                t2 = p1.tile([128, TB], BF16, tag="t2")
                nc.vector.tensor_mul(t2[:], rps[:], sin_t[:, ts_p])
                nc.vector.tensor_add(qkT[:, ft_p, ts_p], t1[:], t2[:])

            for tb in range(NTB):
                ts = slice(tb * TB, (tb + 1) * TB)
                hb = p1h.tile([128, NKT, TB], BF16, tag="hb")
                if tb == 0:
                    # chunked startup loads on the sync HWDGE ring (~0.6us
                    # issue vs ~2us/call SWDGE): first QK matmuls gate on the
                    # first hb/wt chunks instead of the full 5 MB
                    nc.sync.dma_start(hb[:, 0:8, :], hid_r[:, 0:8, ts])
                else:
                    nc.gpsimd.dma_start(hb[:], hid_r[:, :, ts])
                for ft in range(NQK):
                    wt = p1w.tile([128, NKT, 128], BF16, tag="wt")
                    fsl = slice(ft * 128, (ft + 1) * 128)
                    if tb == 0 and ft == 0:
                        nc.sync.dma_start(wt[:, 0:8, :], wqk_r[:, 0:8, fsl])
                        nc.sync.dma_start(wt[:, 8:, :], wqk_r[:, 8:, fsl])
                        for c in range(1, 4):
                            nc.sync.dma_start(hb[:, 8 * c:8 * (c + 1), :],
                                              hid_r[:, 8 * c:8 * (c + 1), ts])
                    else:
                        nc.gpsimd.dma_start(wt[:], wqk_r[:, :, fsl])
                    if tb == 0 and ft == NQK - 1:
                        # wv queued behind tb0's wt loads: needed only by
                        # tb0's trailing V matmuls
                        nc.gpsimd.dma_start(wv_t[:], wv_r[:])
                    ps = ps_qk.tile([128, TB], F32, tag="ps")
                    for kk in range(NKT):
                        nc.tensor.matmul(ps[:], wt[:, kk, :], hb[:, kk, :],
                                         start=(kk == 0), stop=(kk == NKT - 1))
                    raw = p1.tile([128, TB], BF16, tag="raw")
                    nc.scalar.copy(raw[:], ps[:])
                    if tb == 0 and ft == 0:
                        # tail cos/sin rows ride the Scalar HWDGE ring, gated
                        # behind the first RoPE copy so they don't steal HBM
                        # bandwidth from the startup hb/wt loads
                        nc.scalar.dma_start(cos_t[:, TB:], cos[:, TB:])
                        nc.scalar.dma_start(sin_t[:, TB:], sin[:, TB:])
                    emit_rope()
                    rope_pending.append((ps, raw, ft, ts))
                for tt in range(NTT):
                    psv = ps_v.tile([128, cfg.VF], F32, tag="psv")
                    for kk in range(NKT):
                        nc.tensor.matmul(psv[:], hb[:, kk, tt * 128:(tt + 1) * 128],
                                         wv_t[:, kk, :],
                                         start=(kk == 0), stop=(kk == NKT - 1))
                    if tt == 0:
                        emit_rope()
                    # ACT, not DVE: the DVE queue is deep with RoPE muls and
                    # would delay the psv bank release
                    nc.scalar.copy(v_sb[:, tb * NTT + tt, :], psv[:])

        # ---------------- Phases 2+3 ----------------
        with tc.tile_pool(name="res2", bufs=1) as res2:
            attnT = res2.tile([128, cfg.HL, S], BF16, tag="attnT")
            with tc.tile_pool(name="p2", bufs=2) as p2, \
                 tc.tile_pool(name="p2pt", bufs=3) as p2pt, \
                 tc.tile_pool(name="ps_s", bufs=2, space="PSUM") as ps_s, \
                 tc.tile_pool(name="ps_c", bufs=2, space="PSUM") as ps_c, \
                 tc.tile_pool(name="ps_l", bufs=2, space="PSUM") as ps_l:
                # Normalization tail of block X is emitted after block X+1's
                # first EXP, so ACT's FIFO stays clear of the PE-blocking
                # chain (scores -> exp -> AV). lps rows are a 128-partition
                # broadcast of l, so reciprocal+mul need no rebroadcast.
                pending = None

                def flush_tail():
                    nonlocal pending
                    if pending is None:
                        return
                    cps_p, lps_p, h_p, isl_p = pending
                    pending = None
                    rec = p2.tile([128, IB], F32, tag="rec")
                    nc.vector.reciprocal(rec[:], lps_p[:])
                    nc.vector.tensor_mul(attnT[:, h_p, isl_p], cps_p[:], rec[:])

                # Flat group schedule: scores of group g+1 are emitted
                # between exp(g) and AV(g), so the in-order PE queue always
                # has a group of score-matmuls to chew on while ACT finishes
                # exp(g). Diagonal tiles (r = jj - first_diag >= 0) only
                # touch queries i >= 128*r of their block: matmuls run on the
                # [off:IB] sub-range and the causal edge is one [128,128]
                # triangle mask on the first 128-chunk.
                groups = []
                for h in range(cfg.HL):
                    ftk = cfg.HL + (h // (cfg.HL // cfg.KVL))  # k feat-tile
                    hkv = h // (cfg.HL // cfg.KVL)
                    for ib in range(NIB):
                        njt = ND * (ib + 1)
                        for jg in range(njt // GJ):
                            groups.append((h, ib, jg, njt, ftk, hkv))

                def grp_offs(g):
                    h, ib, jg, njt, ftk, hkv = g
                    offs = []
                    for jl in range(GJ):
                        r = jg * GJ + jl - (njt - ND)
                        offs.append(128 * r if r > 0 else 0)
                    return offs

                sps_store = {}

                def emit_scores(gi):
                    h, ib, jg, njt, ftk, hkv = groups[gi]
                    offs = grp_offs(groups[gi])
                    sps = ps_s.tile([128, GJ, IB], F32, tag="sps")
                    for jl in range(GJ):
                        jj = jg * GJ + jl
                        off = offs[jl]
                        nc.tensor.matmul(
                            sps[:, jl, off:IB],
                            qkT[:, ftk, jj * 128:(jj + 1) * 128],
                            qkT[:, h, ib * IB + off:(ib + 1) * IB],
                            start=True, stop=True)
                    sps_store[gi] = (sps, offs)

                emit_scores(0)
                cps = lps = None
                for gi, g in enumerate(groups):
                    h, ib, jg, njt, ftk, hkv = g
                    isl = slice(ib * IB, (ib + 1) * IB)
                    sps, offs = sps_store.pop(gi)
                    pt = p2pt.tile([128, GJ, IB], BF16, tag="pt")
                    if offs[0] > 0:
                        # fully-diagonal group: per-tile exp on the short
                        # sub-ranges beats one grouped call here
                        for jl in range(GJ):
                            off = offs[jl]
                            nc.scalar.activation(
                                pt[:, jl, off:IB], sps[:, jl, off:IB],
                                mybir.ActivationFunctionType.Exp,
                                scale=cfg.scale)
                    else:
                        nc.scalar.activation(
                            pt[:, :, :], sps[:, :, :],
                            mybir.ActivationFunctionType.Exp,
                            scale=cfg.scale)
                    if gi + 1 < len(groups):
                        emit_scores(gi + 1)
                    for jl in range(GJ):
                        jj = jg * GJ + jl
                        if jj - (njt - ND) >= 0:
                            off = offs[jl]
                            nc.vector.tensor_mul(
                                pt[:, jl, off:off + 128],
                                pt[:, jl, off:off + 128], msk_t[:])
                    if jg == 0:
                        cps = ps_c.tile([128, IB], F32, tag="cps")
                        lps = ps_l.tile([128, IB], F32, tag="lps")
                    for jl in range(GJ):
                        jj = jg * GJ + jl
                        off = offs[jl]
                        nc.tensor.matmul(
                            cps[:, off:IB],
                            v_sb[:, jj, hkv * D:(hkv + 1) * D],
                            pt[:, jl, off:IB],
                            start=(jj == 0), stop=(jj == njt - 1),
                            skip_group_check=True)
                    for jl in range(GJ):
                        jj = jg * GJ + jl
                        off = offs[jl]
                        nc.tensor.matmul(
                            lps[:, off:IB], ones128[:],
                            pt[:, jl, off:IB],
                            start=(jj == 0), stop=(jj == njt - 1),
                            skip_group_check=True)
                    if jg == 0:
                        flush_tail()
                    if jg == njt // GJ - 1:
                        pending = (cps, lps, h, isl)
                flush_tail()

            # ------------ Phase 3: o-proj (partial; host all-reduces) ------------
            with tc.tile_pool(name="p3", bufs=2) as p3, \
                 tc.tile_pool(name="p3w", bufs=2) as p3w, \
                 tc.tile_pool(name="ps_o", bufs=8, space="PSUM") as ps_o:
                NOF = cfg.DM // 128
                NKF = cfg.LF // 128
                for of in range(NOF):
                    wt = p3w.tile([128, NKF, 128], BF16, tag="wot")
                    nc.gpsimd.dma_start(wt[:], wo_r[:, :, of * 128:(of + 1) * 128])
                    o_sb = p3.tile([128, S], F32, tag="o_sb")
                    # kf-major: each weight tile stays stationary for all 4
                    # token blocks (4 PSUM banks accumulate in parallel)
                    pss = []
                    for _tb in range(NTB):
                        pso = ps_o.tile([128, TB], F32, tag="pso")
                        pss.append(pso)
                    for kf in range(NKF):
                        for tb in range(NTB):
                            nc.tensor.matmul(
                                pss[tb][:], wt[:, kf, :],
                                attnT[:, kf, tb * TB:(tb + 1) * TB],
                                start=(kf == 0), stop=(kf == NKF - 1),
                                skip_group_check=True)
                    for tb in range(NTB):
                        nc.vector.tensor_copy(o_sb[:, tb * TB:(tb + 1) * TB],
                                              pss[tb][:])
                    nc.gpsimd.dma_start(out[of * 128:(of + 1) * 128, :], o_sb[:])


def shard_inputs(hidden_states, cos, sin, qkv_weight, o_weight, cfg):
    """Host-side shard + transpose + bf16 cast. Returns list of 8 in_maps."""
    S, D, HL, KVL = cfg.S, cfg.D, cfg.HL, cfg.KVL
    H, KV = cfg.H, cfg.KV
    # RoPE tables (identical for both sequences - positions restart)
    cos_t = np.ascontiguousarray(cos[:S].T).astype(np.float32)
    sin_t = np.ascontiguousarray(sin[:S].T).astype(np.float32)
    # causal edge mask: one [128, 128] lower triangle (i >= j), applied to
    # the first 128-chunk of every diagonal tile's live sub-range
    j = np.arange(128)[:, None]
    i = np.arange(128)[None, :]
    masks = (i >= j).astype(BF)
    # signed rotate-half permutation (lhsT layout: rt[d', d] = R[d, d'])
    rtm = np.zeros((128, 128), np.float32)
    half = D // 2
    for d in range(half):
        rtm[half + d, d] = -1.0
        rtm[d, d + half] = 1.0
    rtm = rtm.astype(BF)

    in_maps = []
    for core in range(8):
        b, g = core // cfg.TP, core % cfg.TP
        tok = slice(b * S, (b + 1) * S)
        qr = slice(g * HL * D, (g + 1) * HL * D)
        kr = slice(H * D + g * KVL * D, H * D + (g + 1) * KVL * D)
        vr = slice((H + KV) * D + g * KVL * D, (H + KV) * D + (g + 1) * KVL * D)
        wqk_t = np.ascontiguousarray(
            np.concatenate([qkv_weight[qr], qkv_weight[kr]], 0).T).astype(BF)
        wv_t = np.ascontiguousarray(qkv_weight[vr].T).astype(BF)
        wo_t = np.ascontiguousarray(o_weight[:, qr].T).astype(BF)
        hid_t = np.ascontiguousarray(hidden_states[tok].T).astype(BF)
        in_maps.append({
            "hid_t": hid_t, "wqk_t": wqk_t, "wv_t": wv_t, "wo_t": wo_t,
            "cos_t": cos_t, "sin_t": sin_t, "masks": masks, "rt": rtm,
        })
    return in_maps


def unshard(results, cfg):
    T = cfg.DP * cfg.S
    out = np.zeros((T, cfg.DM), np.float32)
    for core, r in enumerate(results):
        b = core // cfg.TP
        out[b * cfg.S:(b + 1) * cfg.S] += r["out_t"].T
    return out.reshape(1, T, cfg.DM)


def _run(inputs, cfg, trace=False, reps=1):
    import concourse.bacc as bacc
    nc = bacc.Bacc("TRN2", target_bir_lowering=False, debug=False,
                   enable_asserts=False, num_devices=8)
    with tile.TileContext(nc) as tc:
        build_kernel(tc, cfg)
    nc.compile()
    in_maps = shard_inputs(**inputs, cfg=cfg)
    times = []
    res = None
    for _ in range(max(1, reps)):
        res = run_bass_kernel_spmd(nc, in_maps, core_ids=list(range(8)),
                                   trace=trace)
        if res.exec_time_ns is not None:
            times.append(res.exec_time_ns)
    return unshard(res.results, cfg), res, times


def kernel(**inputs):
    out, _, _ = _run(inputs, Cfg())
    return out



# revision 29
# speedup vs baseline: 1.0058x; 1.0058x over previous
"""Llama GQA attention (B=2,S=2048,H=32,KV=8,D=128,DM=4096) on 8 trn2 cores.

Sharding: DP=2 over sequences x TP=4 over heads. Core c = (b=c//4, g=c%4):
seq b's 2048 tokens, q-heads [8g,8g+8), kv-heads [2g,2g+2). Each core computes
its partial o-proj output; host sums the 4 TP partials per sequence.

Device layout trick: everything lives transposed ([feat, tok]) so the
contraction dim is always on partitions and no on-chip transposes are needed.
  qkv^T = W^T.T @ hidden^T          (W^T, hidden^T pre-transposed on host)
  S^T[j,i] = (k^T).T @ q^T          (contraction d=128 = one partition tile)
  P^T = exp(scale*S^T) * mask       (no max-subtraction: scores ~ N(0,1))
  C^T[d,i] = sum_j V[j,d].T ... accumulated as lhsT=V tile, rhs=P^T
  l via all-ones [128,128] lhsT matmul (row-sum broadcast to all
  partitions); 1/l = reciprocal on VectorE (off ACT's exp critical path);
  out^T = Wo^T.T @ (C^T * recip)
RoPE: rotate_half as a signed 128x128 permutation matmul + cos/sin elementwise.
All matmuls bf16 inputs, fp32 PSUM accumulation.

Perf structure (per trace analysis; ~1.13ms -> ~0.87ms):
- all emission is software-pipelined: engine queues are in-order, so the op
  that unblocks the next engine is always emitted before ops that wait
  (scores of group g+1 before AV of g; RoPE tail of ft one group late)
- diagonal attention tiles run on [128*r:IB] sub-ranges (causal triangle),
  with one [128,128] triangle mask; exp stays grouped (ACT call overhead
  costs more than the garbage columns it would skip)
- the softmax tail (reciprocal+mul) of block X is emitted after block X+1's
  first exp so ACT's FIFO never blocks the scores->exp->AV chain
- phase 1 double-buffers hb and rotates 4 QK PSUM banks; startup loads are
  chunked on the sync HWDGE ring so the first matmul starts at ~15us
"""

import numpy as np
import ml_dtypes

import concourse.bass as bass
import concourse.mybir as mybir
import concourse.tile as tile
from concourse.bass_utils import run_bass_kernel_spmd

F32 = mybir.dt.float32
BF16 = mybir.dt.bfloat16
BF = ml_dtypes.bfloat16


class Cfg:
    def __init__(self, S=2048, H=32, KV=8, D=128, TP=4, DP=2, TB=512, IB=512):
        self.S, self.H, self.KV, self.D = S, H, KV, D
        self.TP, self.DP = TP, DP
        self.DM = H * D
        self.HL = H // TP            # local q heads
        self.KVL = KV // TP          # local kv heads
        self.QF = self.HL * D        # local q feats
        self.KF = self.KVL * D
        self.VF = self.KVL * D
        self.LF = self.HL * D        # local o-proj contraction feats
        self.NKT = self.DM // 128    # K-tiles for qkv proj
        self.NQK = (self.QF + self.KF) // 128
        self.TB = min(TB, S)         # token block (qkv / o-proj moving dim)
        self.IB = min(IB, S)         # query block in attention
        self.ND = self.IB // 128     # j-tiles per i-block (diag patterns)
        self.GJ = 2 if self.ND >= 2 else 1   # j-tiles per exp group
        self.scale = float(D) ** -0.5


def build_kernel(tc, cfg):
    nc = tc.nc
    S, D = cfg.S, cfg.D
    TB, IB, ND, GJ = cfg.TB, cfg.IB, cfg.ND, cfg.GJ
    NKT, NQK = cfg.NKT, cfg.NQK
    NTB = S // TB
    NTT = TB // 128                  # tok tiles per block (for V)
    NIB = S // IB

    hid = nc.dram_tensor("hid_t", [cfg.DM, S], BF16, kind="ExternalInput").ap()
    wqk = nc.dram_tensor("wqk_t", [cfg.DM, cfg.QF + cfg.KF], BF16, kind="ExternalInput").ap()
    wv = nc.dram_tensor("wv_t", [cfg.DM, cfg.VF], BF16, kind="ExternalInput").ap()
    wo = nc.dram_tensor("wo_t", [cfg.LF, cfg.DM], BF16, kind="ExternalInput").ap()
    cos = nc.dram_tensor("cos_t", [128, S], F32, kind="ExternalInput").ap()
    sin = nc.dram_tensor("sin_t", [128, S], F32, kind="ExternalInput").ap()
    msk = nc.dram_tensor("masks", [128, 128], BF16, kind="ExternalInput").ap()
    rt = nc.dram_tensor("rt", [128, 128], BF16, kind="ExternalInput").ap()
    out = nc.dram_tensor("out_t", [cfg.DM, S], F32, kind="ExternalOutput").ap()

    hid_r = hid.rearrange("(a p) t -> p a t", p=128)
    wqk_r = wqk.rearrange("(a p) f -> p a f", p=128)
    wv_r = wv.rearrange("(a p) f -> p a f", p=128)
    wo_r = wo.rearrange("(a p) f -> p a f", p=128)

    with tc.tile_pool(name="res", bufs=1) as res:
        qkT = res.tile([128, NQK, S], BF16, tag="qkT")
        v_sb = res.tile([128, S // 128, cfg.VF], BF16, tag="v")
        cos_t = res.tile([128, S], F32, tag="cos")
        sin_t = res.tile([128, S], F32, tag="sin")
        msk_t = res.tile([128, 128], BF16, tag="msk")
        rt_t = res.tile([128, 128], BF16, tag="rt")
        ones128 = res.tile([128, 128], BF16, tag="ones128")

        # Startup: only the first token block's cos/sin rows load up front
        # (the rest loads from the Scalar HWDGE ring once phase 1 is rolling)
        # so hb/wt DMAs own the HBM bandwidth before the first matmul. These
        # ride the scalar ring so the sync ring leads with tb0's hb/wt chunks.
        nc.scalar.dma_start(cos_t[:, 0:TB], cos[:, 0:TB])
        nc.scalar.dma_start(sin_t[:, 0:TB], sin[:, 0:TB])
        nc.scalar.dma_start(msk_t[:], msk[:])
        nc.scalar.dma_start(rt_t[:], rt[:])
        nc.vector.memset(ones128[:], 1.0)

        # Warm-up burst: ~5us of dummy matmuls during the startup DMA wait
        # keeps the HAM clock gate at 8/8 so the first real matmuls run at
        # 2.4 GHz instead of 1.2 (the result is never read; the pool closes
        # so the bank is handed to phase 1 afterwards).
        with tc.tile_pool(name="ps_warm", bufs=1, space="PSUM") as ps_warm:
            warm = ps_warm.tile([128, 128], F32, tag="warm")
            for _ in range(48):
                nc.tensor.matmul(warm[:], ones128[:], ones128[:],
                                 start=True, stop=True)

        # ---------------- Phase 1: fused QKV projection + RoPE ----------------
        with tc.tile_pool(name="p1res", bufs=1) as p1res, \
             tc.tile_pool(name="p1", bufs=3) as p1, \
             tc.tile_pool(name="p1h", bufs=2) as p1h, \
             tc.tile_pool(name="p1w", bufs=2) as p1w, \
             tc.tile_pool(name="ps_qk", bufs=4, space="PSUM") as ps_qk, \
             tc.tile_pool(name="ps_rot", bufs=2, space="PSUM") as ps_rot, \
             tc.tile_pool(name="ps_v", bufs=2, space="PSUM") as ps_v:
            wv_t = p1res.tile([128, NKT, cfg.VF], BF16, tag="wv")

            # RoPE tail of feature-tile ft is emitted one PE-group late, so
            # the rot matmul never heads the PE queue while ACT's raw copy
            # is still in flight.
            rope_pending = []

            def emit_rope():
                if not rope_pending:
                    return
                ps_p, raw_p, ft_p, ts_p = rope_pending.pop()
                rps = ps_rot.tile([128, TB], F32, tag="rps")
                nc.tensor.matmul(rps[:], rt_t[:], raw_p[:], start=True,
                                 stop=True)
                t1 = p1.tile([128, TB], BF16, tag="t1")
                nc.vector.tensor_mul(t1[:], ps_p[:], cos_t[:, ts_p])
                t2 = p1.tile([128, TB], BF16, tag="t2")
                nc.vector.tensor_mul(t2[:], rps[:], sin_t[:, ts_p])
                nc.vector.tensor_add(qkT[:, ft_p, ts_p], t1[:], t2[:])

            for tb in range(NTB):
                ts = slice(tb * TB, (tb + 1) * TB)
                hb = p1h.tile([128, NKT, TB], BF16, tag="hb")
                if tb == 0:
                    # chunked startup loads on the sync HWDGE ring (~0.6us
                    # issue vs ~2us/call SWDGE): first QK matmuls gate on the
                    # first hb/wt chunks instead of the full 5 MB
                    nc.sync.dma_start(hb[:, 0:8, :], hid_r[:, 0:8, ts])
                else:
                    nc.gpsimd.dma_start(hb[:], hid_r[:, :, ts])
                for ft in range(NQK):
                    wt = p1w.tile([128, NKT, 128], BF16, tag="wt")
                    fsl = slice(ft * 128, (ft + 1) * 128)
                    if tb == 0 and ft == 0:
                        nc.sync.dma_start(wt[:, 0:8, :], wqk_r[:, 0:8, fsl])
                        nc.sync.dma_start(wt[:, 8:, :], wqk_r[:, 8:, fsl])
                        for c in range(1, 4):
                            nc.sync.dma_start(hb[:, 8 * c:8 * (c + 1), :],
                                              hid_r[:, 8 * c:8 * (c + 1), ts])
                    else:
                        nc.gpsimd.dma_start(wt[:], wqk_r[:, :, fsl])
                    if tb == 0 and ft == NQK - 1:
                        # wv queued behind tb0's wt loads: needed only by
                        # tb0's trailing V matmuls
                        nc.gpsimd.dma_start(wv_t[:], wv_r[:])
                    ps = ps_qk.tile([128, TB], F32, tag="ps")
                    for kk in range(NKT):
                        nc.tensor.matmul(ps[:], wt[:, kk, :], hb[:, kk, :],
                                         start=(kk == 0), stop=(kk == NKT - 1))
                    raw = p1.tile([128, TB], BF16, tag="raw")
                    nc.scalar.copy(raw[:], ps[:])
                    if tb == 0 and ft == 0:
                        # tail cos/sin rows ride the Scalar HWDGE ring, gated
                        # behind the first RoPE copy so they don't steal HBM
                        # bandwidth from the startup hb/wt loads
                        nc.scalar.dma_start(cos_t[:, TB:], cos[:, TB:])
                        nc.scalar.dma_start(sin_t[:, TB:], sin[:, TB:])
                    emit_rope()
                    rope_pending.append((ps, raw, ft, ts))
                for tt in range(NTT):
                    psv = ps_v.tile([128, cfg.VF], F32, tag="psv")
                    for kk in range(NKT):
                        nc.tensor.matmul(psv[:], hb[:, kk, tt * 128:(tt + 1) * 128],
                                         wv_t[:, kk, :],
                                         start=(kk == 0), stop=(kk == NKT - 1))
                    if tt == 0:
                        emit_rope()
                    # ACT, not DVE: the DVE queue is deep with RoPE muls and
                    # would delay the psv bank release
                    nc.scalar.copy(v_sb[:, tb * NTT + tt, :], psv[:])

        # ---------------- Phases 2+3 ----------------
        with tc.tile_pool(name="res2", bufs=1) as res2:
            attnT = res2.tile([128, cfg.HL, S], BF16, tag="attnT")
            with tc.tile_pool(name="p2", bufs=2) as p2, \
                 tc.tile_pool(name="p2pt", bufs=3) as p2pt, \
                 tc.tile_pool(name="ps_s", bufs=2, space="PSUM") as ps_s, \
                 tc.tile_pool(name="ps_c", bufs=2, space="PSUM") as ps_c, \
                 tc.tile_pool(name="ps_l", bufs=2, space="PSUM") as ps_l:
                # Normalization tail of block X is emitted after block X+1's
                # first EXP, so ACT's FIFO stays clear of the PE-blocking
                # chain (scores -> exp -> AV). lps rows are a 128-partition
                # broadcast of l, so reciprocal+mul need no rebroadcast.
                pending = None

                def flush_tail():
                    nonlocal pending
                    if pending is None:
                        return
                    cps_p, lps_p, h_p, isl_p = pending
                    pending = None
                    rec = p2.tile([128, IB], F32, tag="rec")
                    nc.vector.reciprocal(rec[:], lps_p[:])
                    nc.vector.tensor_mul(attnT[:, h_p, isl_p], cps_p[:], rec[:])

                # Flat group schedule: scores of group g+1 are emitted
                # between exp(g) and AV(g), so the in-order PE queue always
                # has a group of score-matmuls to chew on while ACT finishes
                # exp(g). Diagonal tiles (r = jj - first_diag >= 0) only
                # touch queries i >= 128*r of their block: matmuls run on the
                # [off:IB] sub-range and the causal edge is one [128,128]
                # triangle mask on the first 128-chunk.
                groups = []
                for h in range(cfg.HL):
                    ftk = cfg.HL + (h // (cfg.HL // cfg.KVL))  # k feat-tile
                    hkv = h // (cfg.HL // cfg.KVL)
                    for ib in range(NIB):
                        njt = ND * (ib + 1)
                        for jg in range(njt // GJ):
                            groups.append((h, ib, jg, njt, ftk, hkv))

                def grp_offs(g):
                    h, ib, jg, njt, ftk, hkv = g
                    offs = []
                    for jl in range(GJ):
                        r = jg * GJ + jl - (njt - ND)
                        offs.append(128 * r if r > 0 else 0)
                    return offs

                sps_store = {}

                def emit_scores(gi):
                    h, ib, jg, njt, ftk, hkv = groups[gi]
                    offs = grp_offs(groups[gi])
                    sps = ps_s.tile([128, GJ, IB], F32, tag="sps")
                    for jl in range(GJ):
                        jj = jg * GJ + jl
                        off = offs[jl]
                        nc.tensor.matmul(
                            sps[:, jl, off:IB],
                            qkT[:, ftk, jj * 128:(jj + 1) * 128],
                            qkT[:, h, ib * IB + off:(ib + 1) * IB],
                            start=True, stop=True)
                    sps_store[gi] = (sps, offs)

                emit_scores(0)
                cps = lps = None
                for gi, g in enumerate(groups):
                    h, ib, jg, njt, ftk, hkv = g
                    isl = slice(ib * IB, (ib + 1) * IB)
                    sps, offs = sps_store.pop(gi)
                    pt = p2pt.tile([128, GJ, IB], BF16, tag="pt")
                    if offs[0] > 0:
                        # fully-diagonal group: per-tile exp on the short
                        # sub-ranges beats one grouped call here
                        for jl in range(GJ):
                            off = offs[jl]
                            nc.scalar.activation(
                                pt[:, jl, off:IB], sps[:, jl, off:IB],
                                mybir.ActivationFunctionType.Exp,
                                scale=cfg.scale)
                    else:
                        nc.scalar.activation(
                            pt[:, :, :], sps[:, :, :],
                            mybir.ActivationFunctionType.Exp,
                            scale=cfg.scale)
                    if gi + 1 < len(groups):
                        emit_scores(gi + 1)
                    for jl in range(GJ):
                        jj = jg * GJ + jl
                        if jj - (njt - ND) >= 0:
                            off = offs[jl]
                            nc.vector.tensor_mul(
                                pt[:, jl, off:off + 128],
                                pt[:, jl, off:off + 128], msk_t[:])
                    if jg == 0:
                        cps = ps_c.tile([128, IB], F32, tag="cps")
                        lps = ps_l.tile([128, IB], F32, tag="lps")
                    for jl in range(GJ):
                        jj = jg * GJ + jl
                        off = offs[jl]
                        nc.tensor.matmul(
                            cps[:, off:IB],
                            v_sb[:, jj, hkv * D:(hkv + 1) * D],
                            pt[:, jl, off:IB],
                            start=(jj == 0), stop=(jj == njt - 1),
                            skip_group_check=True)
                    for jl in range(GJ):
                        jj = jg * GJ + jl
                        off = offs[jl]
                        nc.tensor.matmul(
                            lps[:, off:IB], ones128[:],
                            pt[:, jl, off:IB],
                            start=(jj == 0), stop=(jj == njt - 1),
                            skip_group_check=True)
                    if jg == 0:
                        flush_tail()
                    if jg == njt // GJ - 1:
                        pending = (cps, lps, h, isl)
                flush_tail()

            # ------------ Phase 3: o-proj (partial; host all-reduces) ------------
            with tc.tile_pool(name="p3", bufs=2) as p3, \
                 tc.tile_pool(name="p3w", bufs=2) as p3w, \
                 tc.tile_pool(name="ps_o", bufs=8, space="PSUM") as ps_o:
                NOF = cfg.DM // 128
                NKF = cfg.LF // 128
                for of in range(NOF):
                    wt = p3w.tile([128, NKF, 128], BF16, tag="wot")
                    nc.gpsimd.dma_start(wt[:], wo_r[:, :, of * 128:(of + 1) * 128])
                    o_sb = p3.tile([128, S], F32, tag="o_sb")
                    # kf-major: each weight tile stays stationary for all 4
                    # token blocks (4 PSUM banks accumulate in parallel)
                    pss = []
                    for _tb in range(NTB):
                        pso = ps_o.tile([128, TB], F32, tag="pso")
                        pss.append(pso)
                    for kf in range(NKF):
                        for tb in range(NTB):
                            nc.tensor.matmul(
                                pss[tb][:], wt[:, kf, :],
                                attnT[:, kf, tb * TB:(tb + 1) * TB],
                                start=(kf == 0), stop=(kf == NKF - 1),
                                skip_group_check=True)
                    osl = slice(of * 128, (of + 1) * 128)
                    if of == NOF - 1:
                        # last tile: chunked store so the tail DMA only waits
                        # on the final quarter's copy
                        for tb in range(NTB):
                            tsl = slice(tb * TB, (tb + 1) * TB)
                            nc.vector.tensor_copy(o_sb[:, tsl], pss[tb][:])
                            nc.sync.dma_start(out[osl, tsl], o_sb[:, tsl])
                    else:
                        for tb in range(NTB):
                            nc.vector.tensor_copy(o_sb[:, tb * TB:(tb + 1) * TB],
                                                  pss[tb][:])
                        nc.gpsimd.dma_start(out[osl, :], o_sb[:])


def shard_inputs(hidden_states, cos, sin, qkv_weight, o_weight, cfg):
    """Host-side shard + transpose + bf16 cast. Returns list of 8 in_maps."""
    S, D, HL, KVL = cfg.S, cfg.D, cfg.HL, cfg.KVL
    H, KV = cfg.H, cfg.KV
    # RoPE tables (identical for both sequences - positions restart)
    cos_t = np.ascontiguousarray(cos[:S].T).astype(np.float32)
    sin_t = np.ascontiguousarray(sin[:S].T).astype(np.float32)
    # causal edge mask: one [128, 128] lower triangle (i >= j), applied to
    # the first 128-chunk of every diagonal tile's live sub-range
    j = np.arange(128)[:, None]
    i = np.arange(128)[None, :]
    masks = (i >= j).astype(BF)
    # signed rotate-half permutation (lhsT layout: rt[d', d] = R[d, d'])
    rtm = np.zeros((128, 128), np.float32)
    half = D // 2
    for d in range(half):
        rtm[half + d, d] = -1.0
        rtm[d, d + half] = 1.0
    rtm = rtm.astype(BF)

    in_maps = []
    for core in range(8):
        b, g = core // cfg.TP, core % cfg.TP
        tok = slice(b * S, (b + 1) * S)
        qr = slice(g * HL * D, (g + 1) * HL * D)
        kr = slice(H * D + g * KVL * D, H * D + (g + 1) * KVL * D)
        vr = slice((H + KV) * D + g * KVL * D, (H + KV) * D + (g + 1) * KVL * D)
        wqk_t = np.ascontiguousarray(
            np.concatenate([qkv_weight[qr], qkv_weight[kr]], 0).T).astype(BF)
        wv_t = np.ascontiguousarray(qkv_weight[vr].T).astype(BF)
        wo_t = np.ascontiguousarray(o_weight[:, qr].T).astype(BF)
        hid_t = np.ascontiguousarray(hidden_states[tok].T).astype(BF)
        in_maps.append({
            "hid_t": hid_t, "wqk_t": wqk_t, "wv_t": wv_t, "wo_t": wo_t,
            "cos_t": cos_t, "sin_t": sin_t, "masks": masks, "rt": rtm,
        })
    return in_maps


def unshard(results, cfg):
    T = cfg.DP * cfg.S
    out = np.zeros((T, cfg.DM), np.float32)
    for core, r in enumerate(results):
        b = core // cfg.TP
        out[b * cfg.S:(b + 1) * cfg.S] += r["out_t"].T
    return out.reshape(1, T, cfg.DM)


def _run(inputs, cfg, trace=False, reps=1):
    import concourse.bacc as bacc
    nc = bacc.Bacc("TRN2", target_bir_lowering=False, debug=False,
                   enable_asserts=False, num_devices=8)
    with tile.TileContext(nc) as tc:
        build_kernel(tc, cfg)
    nc.compile()
    in_maps = shard_inputs(**inputs, cfg=cfg)
    times = []
    res = None
    for _ in range(max(1, reps)):
        res = run_bass_kernel_spmd(nc, in_maps, core_ids=list(range(8)),
                                   trace=trace)
        if res.exec_time_ns is not None:
            times.append(res.exec_time_ns)
    return unshard(res.results, cfg), res, times


def kernel(**inputs):
    out, _, _ = _run(inputs, Cfg())
    return out



# revision 30
# speedup vs baseline: 1.0202x; 1.0143x over previous
"""Llama GQA attention (B=2,S=2048,H=32,KV=8,D=128,DM=4096) on 8 trn2 cores.

Sharding: DP=2 over sequences x TP=4 over heads. Core c = (b=c//4, g=c%4):
seq b's 2048 tokens, q-heads [8g,8g+8), kv-heads [2g,2g+2). Each core computes
its partial o-proj output; host sums the 4 TP partials per sequence.

Device layout trick: everything lives transposed ([feat, tok]) so the
contraction dim is always on partitions and no on-chip transposes are needed.
  qkv^T = W^T.T @ hidden^T          (W^T, hidden^T pre-transposed on host)
  S^T[j,i] = (k^T).T @ q^T          (contraction d=128 = one partition tile)
  P^T = exp(scale*S^T) * mask       (no max-subtraction: scores ~ N(0,1))
  C^T[d,i] = sum_j V[j,d].T ... accumulated as lhsT=V tile, rhs=P^T
  l via all-ones [128,128] lhsT matmul (row-sum broadcast to all
  partitions); 1/l = reciprocal on VectorE (off ACT's exp critical path);
  out^T = Wo^T.T @ (C^T * recip)
RoPE: rotate_half as a signed 128x128 permutation matmul + cos/sin elementwise.
All matmuls bf16 inputs, fp32 PSUM accumulation.

Perf structure (per trace analysis; ~1.13ms -> ~0.87ms):
- all emission is software-pipelined: engine queues are in-order, so the op
  that unblocks the next engine is always emitted before ops that wait
  (scores of group g+1 before AV of g; RoPE tail of ft one group late)
- diagonal attention tiles run on [128*r:IB] sub-ranges (causal triangle),
  with one [128,128] triangle mask; exp stays grouped (ACT call overhead
  costs more than the garbage columns it would skip)
- the softmax tail (reciprocal+mul) of block X is emitted after block X+1's
  first exp so ACT's FIFO never blocks the scores->exp->AV chain
- phase 1 double-buffers hb and rotates 4 QK PSUM banks; startup loads are
  chunked on the sync HWDGE ring so the first matmul starts at ~15us
"""

import numpy as np
import ml_dtypes

import concourse.bass as bass
import concourse.mybir as mybir
import concourse.tile as tile
from concourse.bass_utils import run_bass_kernel_spmd

F32 = mybir.dt.float32
BF16 = mybir.dt.bfloat16
BF = ml_dtypes.bfloat16


class Cfg:
    def __init__(self, S=2048, H=32, KV=8, D=128, TP=4, DP=2, TB=512, IB=512):
        self.S, self.H, self.KV, self.D = S, H, KV, D
        self.TP, self.DP = TP, DP
        self.DM = H * D
        self.HL = H // TP            # local q heads
        self.KVL = KV // TP          # local kv heads
        self.QF = self.HL * D        # local q feats
        self.KF = self.KVL * D
        self.VF = self.KVL * D
        self.LF = self.HL * D        # local o-proj contraction feats
        self.NKT = self.DM // 128    # K-tiles for qkv proj
        self.NQK = (self.QF + self.KF) // 128
        self.TB = min(TB, S)         # token block (qkv / o-proj moving dim)
        self.IB = min(IB, S)         # query block in attention
        self.ND = self.IB // 128     # j-tiles per i-block (diag patterns)
        self.GJ = 2 if self.ND >= 2 else 1   # j-tiles per exp group
        self.scale = float(D) ** -0.5


def build_kernel(tc, cfg):
    nc = tc.nc
    S, D = cfg.S, cfg.D
    TB, IB, ND, GJ = cfg.TB, cfg.IB, cfg.ND, cfg.GJ
    NKT, NQK = cfg.NKT, cfg.NQK
    NTB = S // TB
    NTT = TB // 128                  # tok tiles per block (for V)
    NIB = S // IB

    hid = nc.dram_tensor("hid_t", [cfg.DM, S], BF16, kind="ExternalInput").ap()
    wqk = nc.dram_tensor("wqk_t", [cfg.DM, cfg.QF + cfg.KF], BF16, kind="ExternalInput").ap()
    wv = nc.dram_tensor("wv_t", [cfg.DM, cfg.VF], BF16, kind="ExternalInput").ap()
    wo = nc.dram_tensor("wo_t", [cfg.LF, cfg.DM], BF16, kind="ExternalInput").ap()
    cos = nc.dram_tensor("cos_t", [128, S], F32, kind="ExternalInput").ap()
    sin = nc.dram_tensor("sin_t", [128, S], F32, kind="ExternalInput").ap()
    msk = nc.dram_tensor("masks", [128, 128], BF16, kind="ExternalInput").ap()
    rt = nc.dram_tensor("rt", [128, 128], BF16, kind="ExternalInput").ap()
    out = nc.dram_tensor("out_t", [cfg.DM, S], F32, kind="ExternalOutput").ap()

    hid_r = hid.rearrange("(a p) t -> p a t", p=128)
    wqk_r = wqk.rearrange("(a p) f -> p a f", p=128)
    wv_r = wv.rearrange("(a p) f -> p a f", p=128)
    wo_r = wo.rearrange("(a p) f -> p a f", p=128)

    with tc.tile_pool(name="res", bufs=1) as res:
        qkT = res.tile([128, NQK, S], BF16, tag="qkT")
        v_sb = res.tile([128, S // 128, cfg.VF], BF16, tag="v")
        cos_t = res.tile([128, S], F32, tag="cos")
        sin_t = res.tile([128, S], F32, tag="sin")
        msk_t = res.tile([128, 128], BF16, tag="msk")
        rt_t = res.tile([128, 128], BF16, tag="rt")
        ones128 = res.tile([128, 128], BF16, tag="ones128")

        # Startup: only the first token block's cos/sin rows load up front
        # (the rest loads from the Scalar HWDGE ring once phase 1 is rolling)
        # so hb/wt DMAs own the HBM bandwidth before the first matmul. These
        # ride the scalar ring so the sync ring leads with tb0's hb/wt chunks.
        nc.scalar.dma_start(cos_t[:, 0:TB], cos[:, 0:TB])
        nc.scalar.dma_start(sin_t[:, 0:TB], sin[:, 0:TB])
        nc.scalar.dma_start(msk_t[:], msk[:])
        nc.scalar.dma_start(rt_t[:], rt[:])
        nc.vector.memset(ones128[:], 1.0)

        # Warm-up burst: ~5us of dummy matmuls during the startup DMA wait
        # keeps the HAM clock gate at 8/8 so the first real matmuls run at
        # 2.4 GHz instead of 1.2 (the result is never read; the pool closes
        # so the bank is handed to phase 1 afterwards).
        with tc.tile_pool(name="ps_warm", bufs=1, space="PSUM") as ps_warm:
            warm = ps_warm.tile([128, 128], F32, tag="warm")
            for _ in range(48):
                nc.tensor.matmul(warm[:], ones128[:], ones128[:],
                                 start=True, stop=True)

        # ---------------- Phase 1: fused QKV projection + RoPE ----------------
        with tc.tile_pool(name="p1res", bufs=1) as p1res, \
             tc.tile_pool(name="p1", bufs=3) as p1, \
             tc.tile_pool(name="p1h", bufs=2) as p1h, \
             tc.tile_pool(name="p1w", bufs=3) as p1w, \
             tc.tile_pool(name="ps_qk", bufs=4, space="PSUM") as ps_qk, \
             tc.tile_pool(name="ps_rot", bufs=2, space="PSUM") as ps_rot, \
             tc.tile_pool(name="ps_v", bufs=2, space="PSUM") as ps_v:
            wv_t = p1res.tile([128, NKT, cfg.VF], BF16, tag="wv")

            # RoPE tail of feature-tile ft is emitted one PE-group late, so
            # the rot matmul never heads the PE queue while ACT's raw copy
            # is still in flight.
            rope_pending = []

            def emit_rope():
                if not rope_pending:
                    return
                ps_p, raw_p, ft_p, ts_p = rope_pending.pop()
                rps = ps_rot.tile([128, TB], F32, tag="rps")
                nc.tensor.matmul(rps[:], rt_t[:], raw_p[:], start=True,
                                 stop=True)
                t1 = p1.tile([128, TB], BF16, tag="t1")
                nc.vector.tensor_mul(t1[:], ps_p[:], cos_t[:, ts_p])
                t2 = p1.tile([128, TB], BF16, tag="t2")
                nc.vector.tensor_mul(t2[:], rps[:], sin_t[:, ts_p])
                nc.vector.tensor_add(qkT[:, ft_p, ts_p], t1[:], t2[:])

            for tb in range(NTB):
                ts = slice(tb * TB, (tb + 1) * TB)
                hb = p1h.tile([128, NKT, TB], BF16, tag="hb")
                if tb == 0:
                    # chunked startup loads on the sync HWDGE ring (~0.6us
                    # issue vs ~2us/call SWDGE): first QK matmuls gate on the
                    # first hb/wt chunks instead of the full 5 MB
                    nc.sync.dma_start(hb[:, 0:8, :], hid_r[:, 0:8, ts])
                else:
                    nc.gpsimd.dma_start(hb[:], hid_r[:, :, ts])
                for ft in range(NQK):
                    wt = p1w.tile([128, NKT, 128], BF16, tag="wt")
                    fsl = slice(ft * 128, (ft + 1) * 128)
                    if tb == 0 and ft == 0:
                        nc.sync.dma_start(wt[:, 0:8, :], wqk_r[:, 0:8, fsl])
                        nc.sync.dma_start(wt[:, 8:, :], wqk_r[:, 8:, fsl])
                        for c in range(1, 4):
                            nc.sync.dma_start(hb[:, 8 * c:8 * (c + 1), :],
                                              hid_r[:, 8 * c:8 * (c + 1), ts])
                    else:
                        nc.gpsimd.dma_start(wt[:], wqk_r[:, :, fsl])
                    if tb == 0 and ft == NQK - 1:
                        # wv queued behind tb0's wt loads: needed only by
                        # tb0's trailing V matmuls
                        nc.gpsimd.dma_start(wv_t[:], wv_r[:])
                    ps = ps_qk.tile([128, TB], F32, tag="ps")
                    for kk in range(NKT):
                        nc.tensor.matmul(ps[:], wt[:, kk, :], hb[:, kk, :],
                                         start=(kk == 0), stop=(kk == NKT - 1))
                    raw = p1.tile([128, TB], BF16, tag="raw")
                    nc.scalar.copy(raw[:], ps[:])
                    if tb == 0 and ft == 0:
                        # tail cos/sin rows ride the Scalar HWDGE ring, gated
                        # behind the first RoPE copy so they don't steal HBM
                        # bandwidth from the startup hb/wt loads
                        nc.scalar.dma_start(cos_t[:, TB:], cos[:, TB:])
                        nc.scalar.dma_start(sin_t[:, TB:], sin[:, TB:])
                    emit_rope()
                    rope_pending.append((ps, raw, ft, ts))
                for tt in range(NTT):
                    psv = ps_v.tile([128, cfg.VF], F32, tag="psv")
                    for kk in range(NKT):
                        nc.tensor.matmul(psv[:], hb[:, kk, tt * 128:(tt + 1) * 128],
                                         wv_t[:, kk, :],
                                         start=(kk == 0), stop=(kk == NKT - 1))
                    if tt == 0:
                        emit_rope()
                    # ACT, not DVE: the DVE queue is deep with RoPE muls and
                    # would delay the psv bank release
                    nc.scalar.copy(v_sb[:, tb * NTT + tt, :], psv[:])

        # ---------------- Phases 2+3 ----------------
        with tc.tile_pool(name="res2", bufs=1) as res2:
            attnT = res2.tile([128, cfg.HL, S], BF16, tag="attnT")
            with tc.tile_pool(name="p2", bufs=2) as p2, \
                 tc.tile_pool(name="p2pt", bufs=3) as p2pt, \
                 tc.tile_pool(name="ps_s", bufs=2, space="PSUM") as ps_s, \
                 tc.tile_pool(name="ps_c", bufs=2, space="PSUM") as ps_c, \
                 tc.tile_pool(name="ps_l", bufs=2, space="PSUM") as ps_l:
                # Normalization tail of block X is emitted after block X+1's
                # first EXP, so ACT's FIFO stays clear of the PE-blocking
                # chain (scores -> exp -> AV). lps rows are a 128-partition
                # broadcast of l, so reciprocal+mul need no rebroadcast.
                pending = None

                def flush_tail():
                    nonlocal pending
                    if pending is None:
                        return
                    cps_p, lps_p, h_p, isl_p = pending
                    pending = None
                    rec = p2.tile([128, IB], F32, tag="rec")
                    nc.vector.reciprocal(rec[:], lps_p[:])
                    nc.vector.tensor_mul(attnT[:, h_p, isl_p], cps_p[:], rec[:])

                # Flat group schedule: scores of group g+1 are emitted
                # between exp(g) and AV(g), so the in-order PE queue always
                # has a group of score-matmuls to chew on while ACT finishes
                # exp(g). Diagonal tiles (r = jj - first_diag >= 0) only
                # touch queries i >= 128*r of their block: matmuls run on the
                # [off:IB] sub-range and the causal edge is one [128,128]
                # triangle mask on the first 128-chunk.
                groups = []
                for h in range(cfg.HL):
                    ftk = cfg.HL + (h // (cfg.HL // cfg.KVL))  # k feat-tile
                    hkv = h // (cfg.HL // cfg.KVL)
                    for ib in range(NIB):
                        njt = ND * (ib + 1)
                        for jg in range(njt // GJ):
                            groups.append((h, ib, jg, njt, ftk, hkv))

                def grp_offs(g):
                    h, ib, jg, njt, ftk, hkv = g
                    offs = []
                    for jl in range(GJ):
                        r = jg * GJ + jl - (njt - ND)
                        offs.append(128 * r if r > 0 else 0)
                    return offs

                sps_store = {}

                def emit_scores(gi):
                    h, ib, jg, njt, ftk, hkv = groups[gi]
                    offs = grp_offs(groups[gi])
                    sps = ps_s.tile([128, GJ, IB], F32, tag="sps")
                    for jl in range(GJ):
                        jj = jg * GJ + jl
                        off = offs[jl]
                        nc.tensor.matmul(
                            sps[:, jl, off:IB],
                            qkT[:, ftk, jj * 128:(jj + 1) * 128],
                            qkT[:, h, ib * IB + off:(ib + 1) * IB],
                            start=True, stop=True)
                    sps_store[gi] = (sps, offs)

                emit_scores(0)
                cps = lps = None
                for gi, g in enumerate(groups):
                    h, ib, jg, njt, ftk, hkv = g
                    isl = slice(ib * IB, (ib + 1) * IB)
                    sps, offs = sps_store.pop(gi)
                    pt = p2pt.tile([128, GJ, IB], BF16, tag="pt")
                    if offs[0] > 0:
                        # fully-diagonal group: per-tile exp on the short
                        # sub-ranges beats one grouped call here
                        for jl in range(GJ):
                            off = offs[jl]
                            nc.scalar.activation(
                                pt[:, jl, off:IB], sps[:, jl, off:IB],
                                mybir.ActivationFunctionType.Exp,
                                scale=cfg.scale)
                    else:
                        nc.scalar.activation(
                            pt[:, :, :], sps[:, :, :],
                            mybir.ActivationFunctionType.Exp,
                            scale=cfg.scale)
                    if gi + 1 < len(groups):
                        emit_scores(gi + 1)
                    for jl in range(GJ):
                        jj = jg * GJ + jl
                        if jj - (njt - ND) >= 0:
                            off = offs[jl]
                            nc.vector.tensor_mul(
                                pt[:, jl, off:off + 128],
                                pt[:, jl, off:off + 128], msk_t[:])
                    if jg == 0:
                        cps = ps_c.tile([128, IB], F32, tag="cps")
                        lps = ps_l.tile([128, IB], F32, tag="lps")
                    for jl in range(GJ):
                        jj = jg * GJ + jl
                        off = offs[jl]
                        nc.tensor.matmul(
                            cps[:, off:IB],
                            v_sb[:, jj, hkv * D:(hkv + 1) * D],
                            pt[:, jl, off:IB],
                            start=(jj == 0), stop=(jj == njt - 1),
                            skip_group_check=True)
                    for jl in range(GJ):
                        jj = jg * GJ + jl
                        off = offs[jl]
                        nc.tensor.matmul(
                            lps[:, off:IB], ones128[:],
                            pt[:, jl, off:IB],
                            start=(jj == 0), stop=(jj == njt - 1),
                            skip_group_check=True)
                    if jg == 0:
                        flush_tail()
                    if jg == njt // GJ - 1:
                        pending = (cps, lps, h, isl)
                flush_tail()

            # ------------ Phase 3: o-proj (partial; host all-reduces) ------------
            with tc.tile_pool(name="p3", bufs=2) as p3, \
                 tc.tile_pool(name="p3w", bufs=2) as p3w, \
                 tc.tile_pool(name="ps_o", bufs=8, space="PSUM") as ps_o:
                NOF = cfg.DM // 128
                NKF = cfg.LF // 128
                for of in range(NOF):
                    wt = p3w.tile([128, NKF, 128], BF16, tag="wot")
                    nc.gpsimd.dma_start(wt[:], wo_r[:, :, of * 128:(of + 1) * 128])
                    o_sb = p3.tile([128, S], F32, tag="o_sb")
                    # kf-major: each weight tile stays stationary for all 4
                    # token blocks (4 PSUM banks accumulate in parallel)
                    pss = []
                    for _tb in range(NTB):
                        pso = ps_o.tile([128, TB], F32, tag="pso")
                        pss.append(pso)
                    for kf in range(NKF):
                        for tb in range(NTB):
                            nc.tensor.matmul(
                                pss[tb][:], wt[:, kf, :],
                                attnT[:, kf, tb * TB:(tb + 1) * TB],
                                start=(kf == 0), stop=(kf == NKF - 1),
                                skip_group_check=True)
                    osl = slice(of * 128, (of + 1) * 128)
                    if of == NOF - 1:
                        # last tile: chunked store so the tail DMA only waits
                        # on the final quarter's copy
                        for tb in range(NTB):
                            tsl = slice(tb * TB, (tb + 1) * TB)
                            nc.vector.tensor_copy(o_sb[:, tsl], pss[tb][:])
                            nc.sync.dma_start(out[osl, tsl], o_sb[:, tsl])
                    else:
                        for tb in range(NTB):
                            nc.vector.tensor_copy(o_sb[:, tb * TB:(tb + 1) * TB],
                                                  pss[tb][:])
                        nc.gpsimd.dma_start(out[osl, :], o_sb[:])


def shard_inputs(hidden_states, cos, sin, qkv_weight, o_weight, cfg):
    """Host-side shard + transpose + bf16 cast. Returns list of 8 in_maps."""
    S, D, HL, KVL = cfg.S, cfg.D, cfg.HL, cfg.KVL
    H, KV = cfg.H, cfg.KV
    # RoPE tables (identical for both sequences - positions restart)
    cos_t = np.ascontiguousarray(cos[:S].T).astype(np.float32)
    sin_t = np.ascontiguousarray(sin[:S].T).astype(np.float32)
    # causal edge mask: one [128, 128] lower triangle (i >= j), applied to
    # the first 128-chunk of every diagonal tile's live sub-range
    j = np.arange(128)[:, None]
    i = np.arange(128)[None, :]
    masks = (i >= j).astype(BF)
    # signed rotate-half permutation (lhsT layout: rt[d', d] = R[d, d'])
    rtm = np.zeros((128, 128), np.float32)
    half = D // 2
    for d in range(half):
        rtm[half + d, d] = -1.0
        rtm[d, d + half] = 1.0
    rtm = rtm.astype(BF)

    in_maps = []
    for core in range(8):
        b, g = core // cfg.TP, core % cfg.TP
        tok = slice(b * S, (b + 1) * S)
        qr = slice(g * HL * D, (g + 1) * HL * D)
        kr = slice(H * D + g * KVL * D, H * D + (g + 1) * KVL * D)
        vr = slice((H + KV) * D + g * KVL * D, (H + KV) * D + (g + 1) * KVL * D)
        wqk_t = np.ascontiguousarray(
            np.concatenate([qkv_weight[qr], qkv_weight[kr]], 0).T).astype(BF)
        wv_t = np.ascontiguousarray(qkv_weight[vr].T).astype(BF)
        wo_t = np.ascontiguousarray(o_weight[:, qr].T).astype(BF)
        hid_t = np.ascontiguousarray(hidden_states[tok].T).astype(BF)
        in_maps.append({
            "hid_t": hid_t, "wqk_t": wqk_t, "wv_t": wv_t, "wo_t": wo_t,
            "cos_t": cos_t, "sin_t": sin_t, "masks": masks, "rt": rtm,
        })
    return in_maps


def unshard(results, cfg):
    T = cfg.DP * cfg.S
    out = np.zeros((T, cfg.DM), np.float32)
    for core, r in enumerate(results):
        b = core // cfg.TP
        out[b * cfg.S:(b + 1) * cfg.S] += r["out_t"].T
    return out.reshape(1, T, cfg.DM)


def _run(inputs, cfg, trace=False, reps=1):
    import concourse.bacc as bacc
    nc = bacc.Bacc("TRN2", target_bir_lowering=False, debug=False,
                   enable_asserts=False, num_devices=8)
    with tile.TileContext(nc) as tc:
        build_kernel(tc, cfg)
    nc.compile()
    in_maps = shard_inputs(**inputs, cfg=cfg)
    times = []
    res = None
    for _ in range(max(1, reps)):
        res = run_bass_kernel_spmd(nc, in_maps, core_ids=list(range(8)),
                                   trace=trace)
        if res.exec_time_ns is not None:
            times.append(res.exec_time_ns)
    return unshard(res.results, cfg), res, times


def kernel(**inputs):
    out, _, _ = _run(inputs, Cfg())
    return out

